# revision 1
# baseline (speedup 1.0000x reference)
"""GRU-D decoder kernel for Trainium2 (8 NeuronCores, data-parallel over batch).

Math (mask == ones everywhere, which the reference hardcodes):
  x_hat = C (constant), d = dt broadcast, gamma_x unused.
  gamma[t,b,j] = exp(-relu(dt[t,b] * colsum(Wgh)[j] + bgh[j]))   (precomputed host-side)
  per step: hdec = gamma_t * h
            z = sigmoid(hdec @ Wz_h + Az0);  r = sigmoid(hdec @ Wr_h + Ar0)
            htl = tanh((r*hdec) @ Wh_h + Ah0)
            h = hdec + z*(htl - hdec)
  out[t] = h_t @ Wlin + blin
  where A?0 = C @ W?_x + colsum(W?_m) + b?  (time-constant, precomputed host-side).

Device layout: everything transposed (H on partitions as 4 tiles of 128,
batch=64 on the free dim), packed as SBUF tiles (128, 4*64) with column
index = kt*64 + b.  Gate matmuls use the weight blocks as stationary
operands and hdec slices as moving operands; outputs land natively in the
same transposed layout, so no transposes are needed anywhere.  The
per-step tail (tanh/blend/decay) is split into two column halves so the
tensor engine can start the next group while the tail of the previous
half is still on Scalar/Vector.
"""

import numpy as np
import ml_dtypes

T, B, H, O = 100, 512, 512, 512
NCORES = 8
BL = B // NCORES  # 64
KC = 4  # contraction chunks of 128
JT = 4  # output j-tiles of 128
FR = JT * BL  # 256
HB = FR // 2  # 128 (half of the free dim; = 2 j-tiles)
GCH = 20  # gamma chunk (steps per DMA)

_BUILD_CACHE = {}


def _build_program():
    if "nc" in _BUILD_CACHE:
        return _BUILD_CACHE["nc"]

    import concourse.tile as tile
    import concourse.mybir as mybir
    from concourse import bacc
    from contextlib import ExitStack

    f32 = mybir.dt.float32
    bf16 = mybir.dt.bfloat16
    AF = mybir.ActivationFunctionType

    nc = bacc.Bacc("TRN2", target_bir_lowering=False, debug=False,
                   num_devices=NCORES)

    gam_d = nc.dram_tensor("gam", [128, T, FR], f32, kind="ExternalInput")
    wzr_d = nc.dram_tensor("wzr", [128, KC * 2 * JT * 128], bf16, kind="ExternalInput")
    wht_d = nc.dram_tensor("wht", [128, KC * JT * 128], bf16, kind="ExternalInput")
    wlin_d = nc.dram_tensor("wlin", [128, KC * O], bf16, kind="ExternalInput")
    a0z_d = nc.dram_tensor("a0z", [128, FR], bf16, kind="ExternalInput")
    a0r_d = nc.dram_tensor("a0r", [128, FR], bf16, kind="ExternalInput")
    a0h_d = nc.dram_tensor("a0h", [128, FR], bf16, kind="ExternalInput")
    ident_d = nc.dram_tensor("ident", [128, 128], bf16, kind="ExternalInput")
    ones_d = nc.dram_tensor("ones64", [1, BL], bf16, kind="ExternalInput")
    blinr_d = nc.dram_tensor("blinr", [1, O], bf16, kind="ExternalInput")
    out_d = nc.dram_tensor("out", [T, BL, O], f32, kind="ExternalOutput")

    with tile.TileContext(nc) as tc, ExitStack() as ctx:
        constp = ctx.enter_context(tc.tile_pool(name="const", bufs=1))
        gpool = ctx.enter_context(tc.tile_pool(name="gam", bufs=2))
        statep = ctx.enter_context(tc.tile_pool(name="state", bufs=1))
        hdp = ctx.enter_context(tc.tile_pool(name="hd", bufs=2))
        actp = ctx.enter_context(tc.tile_pool(name="act", bufs=2))
        pzp = ctx.enter_context(tc.tile_pool(name="pz", bufs=1, space="PSUM"))
        prp = ctx.enter_context(tc.tile_pool(name="pr", bufs=1, space="PSUM"))
        php0 = ctx.enter_context(tc.tile_pool(name="ph0", bufs=1, space="PSUM"))
        php1 = ctx.enter_context(tc.tile_pool(name="ph1", bufs=1, space="PSUM"))
        pjp = ctx.enter_context(tc.tile_pool(name="pj", bufs=2, space="PSUM"))

        wzr = constp.tile([128, KC * 2 * JT * 128], bf16)
        nc.sync.dma_start(wzr[:], wzr_d[:])
        wht = constp.tile([128, KC * JT * 128], bf16)
        nc.sync.dma_start(wht[:], wht_d[:])
        wlin = constp.tile([128, KC * O], bf16)
        nc.sync.dma_start(wlin[:], wlin_d[:])
        a0z = constp.tile([128, FR], bf16)
        nc.sync.dma_start(a0z[:], a0z_d[:])
        a0r = constp.tile([128, FR], bf16)
        nc.sync.dma_start(a0r[:], a0r_d[:])
        a0h = constp.tile([128, FR], bf16)
        nc.sync.dma_start(a0h[:], a0h_d[:])
        ident = constp.tile([128, 128], bf16)
        nc.sync.dma_start(ident[:], ident_d[:])
        ones64 = constp.tile([1, BL], bf16)
        nc.sync.dma_start(ones64[:], ones_d[:])
        blinr = constp.tile([1, O], bf16)
        nc.sync.dma_start(blinr[:], blinr_d[:])

        h = statep.tile([128, FR], f32)
        nc.vector.memset(h[:], 0.0)

        def wzr_blk(g, jo, kc):
            i = ((kc * 2 + g) * JT + jo) * 128
            return wzr[:, i:i + 128]

        def wht_blk(jo, kc):
            i = (kc * JT + jo) * 128
            return wht[:, i:i + 128]

        # gamma chunks, preloaded half a chunk ahead
        chunks = {}

        def ensure_chunk(c):
            if c in chunks or c * GCH >= T:
                return
            t0 = c * GCH
            t1 = min(t0 + GCH, T)
            gt = gpool.tile([128, GCH * FR], f32, tag="gchunk")
            nc.sync.dma_start(gt[:, 0:(t1 - t0) * FR], gam_d[:, t0:t1, :])
            chunks[c] = gt

        def gamma_half(tt, hf):
            c2, o2 = divmod(tt, GCH)
            return chunks[c2][:, o2 * FR + hf * HB: o2 * FR + (hf + 1) * HB]

        ensure_chunk(0)

        # step-0 decayed state is zero
        hdf = hdp.tile([128, FR], f32, tag="hdf")
        nc.vector.memset(hdf[:], 0.0)
        hdb = hdp.tile([128, FR], bf16, tag="hdb")
        nc.vector.memset(hdb[:], 0.0)

        hbf_prev = None
        pj_prev = None

        for t in range(T):
            c, o = divmod(t, GCH)
            if o == GCH // 2:
                ensure_chunk(c + 1)

            # ---- output DMA for step t-1 (projection ran at the end of t-1)
            if pj_prev is not None:
                osb = actp.tile([BL, O], f32, tag="osb")
                nc.scalar.copy(osb[:], pj_prev[:])
                nc.sync.dma_start(out_d[t - 1], osb[:])

            # ---- r gate matmuls, jo-major: each pr j-slice completes after 4
            # MMs so sigmoid(r) halves start while later slices still run
            pr = prp.tile([128, FR], f32, tag="pr")
            nc.tensor.matmul(pr[:], ident[:], a0r[:], start=True, stop=False)
            for jo in range(JT):
                for kc in range(KC):
                    nc.tensor.matmul(
                        pr[:, jo * BL:(jo + 1) * BL],
                        wzr_blk(1, jo, kc),
                        hdb[:, kc * BL:(kc + 1) * BL],
                        start=False, stop=(kc == KC - 1),
                    )
            rb = actp.tile([128, FR], bf16, tag="rb")
            nc.scalar.activation(rb[:, 0:HB], pr[:, 0:HB], AF.Sigmoid)
            nc.scalar.activation(rb[:, HB:FR], pr[:, HB:FR], AF.Sigmoid)
            rh = hdp.tile([128, FR], bf16, tag="rh")
            nc.vector.tensor_mul(rh[:, 0:HB], rb[:, 0:HB], hdb[:, 0:HB])
            nc.vector.tensor_mul(rh[:, HB:FR], rb[:, HB:FR], hdb[:, HB:FR])

            # ---- z gate first half (jo 0,1)
            pz = pzp.tile([128, FR], f32, tag="pz")
            nc.tensor.matmul(pz[:], ident[:], a0z[:], start=True, stop=False)
            for jo in (0, 1):
                for kc in range(KC):
                    nc.tensor.matmul(
                        pz[:, jo * BL:(jo + 1) * BL],
                        wzr_blk(0, jo, kc),
                        hdb[:, kc * BL:(kc + 1) * BL],
                        start=False, stop=(kc == KC - 1),
                    )

            # ---- candidate gate, kc-chunks 0,1 (gated only by rh half 0)
            ph0 = php0.tile([128, HB], f32, tag="ph0")
            ph1 = php1.tile([128, HB], f32, tag="ph1")
            nc.tensor.matmul(ph0[:], ident[:], a0h[:, 0:HB], start=True, stop=False)
            nc.tensor.matmul(ph1[:], ident[:], a0h[:, HB:FR], start=True, stop=False)
            for kc in (0, 1):
                for jo in range(JT):
                    tgt = ph0 if jo < 2 else ph1
                    nc.tensor.matmul(
                        tgt[:, (jo % 2) * BL:(jo % 2 + 1) * BL],
                        wht_blk(jo, kc),
                        rh[:, kc * BL:(kc + 1) * BL],
                        start=False, stop=False,
                    )

            # ---- z gate second half (jo 2,3)
            for jo in (2, 3):
                for kc in range(KC):
                    nc.tensor.matmul(
                        pz[:, jo * BL:(jo + 1) * BL],
                        wzr_blk(0, jo, kc),
                        hdb[:, kc * BL:(kc + 1) * BL],
                        start=False, stop=(kc == KC - 1),
                    )
            zf = actp.tile([128, FR], f32, tag="zf")
            nc.scalar.activation(zf[:, 0:HB], pz[:, 0:HB], AF.Sigmoid)
            nc.scalar.activation(zf[:, HB:FR], pz[:, HB:FR], AF.Sigmoid)

            # ---- candidate gate, kc-chunks 2,3; jo 0,1 slices finish first so
            # tanh(half 0) can start while jo 2,3 still accumulate
            for jo in (0, 1):
                nc.tensor.matmul(
                    ph0[:, jo * BL:(jo + 1) * BL], wht_blk(jo, 2),
                    rh[:, 2 * BL:3 * BL], start=False, stop=False)
                nc.tensor.matmul(
                    ph0[:, jo * BL:(jo + 1) * BL], wht_blk(jo, 3),
                    rh[:, 3 * BL:4 * BL], start=False, stop=True)
            for jo in (2, 3):
                nc.tensor.matmul(
                    ph1[:, (jo - 2) * BL:(jo - 1) * BL], wht_blk(jo, 2),
                    rh[:, 2 * BL:3 * BL], start=False, stop=False)
                nc.tensor.matmul(
                    ph1[:, (jo - 2) * BL:(jo - 1) * BL], wht_blk(jo, 3),
                    rh[:, 3 * BL:4 * BL], start=False, stop=True)

            # ---- blend: h = (1-z)*hdec + z*htl, with (1-z)*hdec computed
            # off the tanh critical path
            zm = actp.tile([128, FR], f32, tag="zm")
            nc.vector.tensor_scalar(zm[:, 0:HB], zf[:, 0:HB], -1.0, 1.0,
                                    mybir.AluOpType.mult, mybir.AluOpType.add)
            pp_ = actp.tile([128, FR], f32, tag="pp")
            nc.vector.tensor_mul(pp_[:, 0:HB], zm[:, 0:HB], hdf[:, 0:HB])
            nc.vector.tensor_scalar(zm[:, HB:FR], zf[:, HB:FR], -1.0, 1.0,
                                    mybir.AluOpType.mult, mybir.AluOpType.add)
            nc.vector.tensor_mul(pp_[:, HB:FR], zm[:, HB:FR], hdf[:, HB:FR])

            hdf_n = hdb_n = None
            if t + 1 < T:
                hdf_n = hdp.tile([128, FR], f32, tag="hdf")
                hdb_n = hdp.tile([128, FR], bf16, tag="hdb")
            for hf, ph in ((0, ph0), (1, ph1)):
                sl = slice(hf * HB, (hf + 1) * HB)
                htl = actp.tile([128, HB], f32, tag=f"htl{hf}")
                nc.scalar.activation(htl[:], ph[:], AF.Tanh)
                qq = actp.tile([128, HB], f32, tag=f"qq{hf}")
                nc.vector.tensor_mul(qq[:], zf[:, sl], htl[:])
                nc.vector.tensor_add(h[:, sl], qq[:], pp_[:, sl])
                if t + 1 < T:
                    # bf16 decayed state straight from the fp32 mul (cast on write)
                    nc.vector.tensor_mul(hdb_n[:, sl], gamma_half(t + 1, hf), h[:, sl])
            if t + 1 < T:
                nc.vector.tensor_mul(hdf_n[:], chunks[(t + 1) // GCH][
                    :, ((t + 1) % GCH) * FR:((t + 1) % GCH + 1) * FR], h[:])
                hdf, hdb = hdf_n, hdb_n

            hbf = actp.tile([128, FR], bf16, tag="hbf")
            nc.scalar.copy(hbf[:], h[:])
            hbf_prev = hbf

            # ---- projection of h(t) at the end of the PE stream (fills the
            # tanh/blend tail); DMA'd out at the start of step t+1
            pj_prev = pjp.tile([BL, O], f32, tag="pj")
            nc.tensor.matmul(pj_prev[:], ones64[:], blinr[:], start=True, stop=False)
            for kc in range(KC):
                nc.tensor.matmul(
                    pj_prev[:],
                    hbf_prev[:, kc * BL:(kc + 1) * BL],
                    wlin[:, kc * O:(kc + 1) * O],
                    start=False, stop=(kc == KC - 1),
                )

        osb = actp.tile([BL, O], f32, tag="osb")
        nc.scalar.copy(osb[:], pj_prev[:])
        nc.sync.dma_start(out_d[T - 1], osb[:])

    nc.compile()
    _BUILD_CACHE["nc"] = nc
    return nc


def _host_prep(C, t, Wz, bz, Wr, br, Wh, bh, Wgh, bgh, Wlin, blin):
    """Build per-core input maps (all the precomputed, packed device tensors)."""
    bf = ml_dtypes.bfloat16

    s = Wgh.sum(axis=0)  # (H,)
    t3 = t[:, :, 0]  # (T,B)
    dt = np.concatenate([np.zeros((1, B), np.float32), t3[1:] - t3[:-1]], axis=0)
    # gamma (T,B,H) fp32
    gam = np.exp(-np.maximum(dt[:, :, None] * s[None, None, :] + bgh[None, None, :], 0.0)).astype(np.float32)

    def gate_const(W, b):
        # C @ W_x + colsum(W_m) + b  -> (B,H)
        return C @ W[0:H] + (W[2 * H:3 * H].sum(axis=0) + b)[None, :]

    Az0 = gate_const(Wz, bz).astype(np.float32)
    Ar0 = gate_const(Wr, br).astype(np.float32)
    Ah0 = gate_const(Wh, bh).astype(np.float32)

    Wg = np.stack([Wz[H:2 * H], Wr[H:2 * H]])  # (2,H,H)
    # wzr packed: [k, (kc,g,jo,m)]
    wzr = Wg.reshape(2, KC, 128, JT, 128).transpose(2, 1, 0, 3, 4).reshape(128, KC * 2 * JT * 128)
    wht = Wh[H:2 * H].reshape(KC, 128, JT, 128).transpose(1, 0, 2, 3).reshape(128, KC * JT * 128)
    wlin = Wlin.reshape(KC, 128, O).transpose(1, 0, 2).reshape(128, KC * O)
    wzr = np.ascontiguousarray(wzr, dtype=bf)
    wht = np.ascontiguousarray(wht, dtype=bf)
    wlin = np.ascontiguousarray(wlin, dtype=bf)
    ident = np.eye(128, dtype=bf)

    in_maps = []
    for i in range(NCORES):
        sl = slice(i * BL, (i + 1) * BL)
        gf = gam[:, sl, :]  # (T,BL,H)
        # gam packed: [p, t, kt*BL+b]
        gp = np.ascontiguousarray(gf.reshape(T, BL, KC, 128).transpose(3, 0, 2, 1).reshape(128, T, KC * BL))

        def packA(A):
            return np.ascontiguousarray(
                A[sl].reshape(BL, JT, 128).transpose(2, 1, 0).reshape(128, JT * BL), dtype=bf)

        in_maps.append({
            "gam": gp,
            "wzr": wzr,
            "wht": wht,
            "wlin": wlin,
            "a0z": packA(Az0),
            "a0r": packA(Ar0),
            "a0h": packA(Ah0),
            "ident": ident,
            "ones64": np.ones((1, BL), dtype=bf),
            "blinr": np.ascontiguousarray(blin.reshape(1, O), dtype=bf),
        })
    return in_maps


def kernel(C, t, mask, Wz, bz, Wr, br, Wh, bh, Wgh, bgh, wgx, bgx, Wlin, blin,
           _trace=False, _trace_kwargs=None):
    C = np.asarray(C, np.float32)
    t = np.asarray(t, np.float32)
    nc = _build_program()
    in_maps = _host_prep(C, t,
                         np.asarray(Wz, np.float32), np.asarray(bz, np.float32),
                         np.asarray(Wr, np.float32), np.asarray(br, np.float32),
                         np.asarray(Wh, np.float32), np.asarray(bh, np.float32),
                         np.asarray(Wgh, np.float32), np.asarray(bgh, np.float32),
                         np.asarray(Wlin, np.float32), np.asarray(blin, np.float32))

    from concourse.bass_utils import run_bass_kernel_spmd
    res = run_bass_kernel_spmd(nc, in_maps, list(range(NCORES)),
                               trace=_trace, **(_trace_kwargs or {}))
    outs = [res.results[i]["out"] for i in range(NCORES)]
    full = np.concatenate(outs, axis=1).astype(np.float32)  # (T,B,O)
    kernel._last_results = res
    return full



# revision 6
# speedup vs baseline: 1.1394x; 1.1394x over previous
"""GRU-D decoder kernel for Trainium2 (8 NeuronCores, data-parallel over batch).

Math (mask == ones everywhere, which the reference hardcodes):
  x_hat = C (constant), d = dt broadcast, gamma_x unused.
  gamma[t,b,j] = exp(-relu(dt[t,b] * colsum(Wgh)[j] + bgh[j]))   (precomputed host-side)
  per step: hdec = gamma_t * h
            z = sigmoid(hdec @ Wz_h + Az0);  r = sigmoid(hdec @ Wr_h + Ar0)
            htl = tanh((r*hdec) @ Wh_h + Ah0)
            h = hdec + z*(htl - hdec)
  out[t] = h_t @ Wlin + blin
  where A?0 = C @ W?_x + colsum(W?_m) + b?  (time-constant, precomputed host-side).

Device layout: transposed (H on partitions as 4 tiles of 128, batch=64 on the
free dim), packed as SBUF tiles (128, 4*64) with column index = kt*64 + b.

v2 design vs the first working version:
  * z and r share one PSUM bank [128,512] (r cols 0:256, z cols 256:512) with a
    single fused bias-init matmul (ident @ [a0r|a0z]).
  * The output projection is batched over 8-step chunks: h is archived (bf16)
    into an SBUF ring by GpSimd, then projected with wlin 128x128 stationary
    blocks and 512-wide moving operands; blin is added by GpSimd
    (tensor_single_scalar) on the PSUM->SBUF copy, so no bias matmul and no
    half-array matmuls.  Chunk pieces are drip-fed one per step boundary as
    dependency-free PE filler to keep the tensor engine busy (p-state!) while
    the recurrent tail (tanh/blend/decay) completes.
  * Gate matmul waves are ordered to minimize the recurrent dependency chain:
    r runs (kc0,kc1)xall-jo then (kc2,kc3) by jo pairs so sigmoid(r) halves
    start as early as possible; candidate runs kc-major with the half-0 stop
    first so tanh half 0 starts before the last candidate matmuls finish.
  * Output is written [O, T, BL] per core (partition-major) so DMA lines are
    1KB contiguous; the host transposes back.
"""

import numpy as np
import ml_dtypes

T, B, H, O = 100, 512, 512, 512
NCORES = 8
BL = B // NCORES  # 64
KC = 4   # contraction chunks of 128
JT = 4   # output j-tiles of 128
FR = JT * BL  # 256
HB = FR // 2  # 128
GCH = 20  # gamma chunk (steps per DMA)
CH = 8    # projection chunk (steps)

_BUILD_CACHE = {}


def _build_program():
    if "nc" in _BUILD_CACHE:
        return _BUILD_CACHE["nc"]

    import concourse.tile as tile
    import concourse.mybir as mybir
    from concourse import bacc
    from contextlib import ExitStack

    f32 = mybir.dt.float32
    bf16 = mybir.dt.bfloat16
    AF = mybir.ActivationFunctionType

    nc = bacc.Bacc("TRN2", target_bir_lowering=False, debug=False,
                   num_devices=NCORES)

    gam_d = nc.dram_tensor("gam", [128, T, FR], f32, kind="ExternalInput")
    wzr_d = nc.dram_tensor("wzr", [128, KC * 2 * JT * 128], bf16, kind="ExternalInput")
    wht_d = nc.dram_tensor("wht", [128, KC * JT * 128], bf16, kind="ExternalInput")
    wlin_d = nc.dram_tensor("wlin", [128, KC * JT * 128], bf16, kind="ExternalInput")
    a0zr_d = nc.dram_tensor("a0zr", [128, 2 * FR], bf16, kind="ExternalInput")
    a0h_d = nc.dram_tensor("a0h", [128, FR], bf16, kind="ExternalInput")
    ident_d = nc.dram_tensor("ident", [128, 128], bf16, kind="ExternalInput")
    blinc_d = nc.dram_tensor("blinc", [128, JT], f32, kind="ExternalInput")
    out_d = nc.dram_tensor("out", [O, T, BL], f32, kind="ExternalOutput")

    with tile.TileContext(nc) as tc, ExitStack() as ctx:
        constp = ctx.enter_context(tc.tile_pool(name="const", bufs=1))
        gpool = ctx.enter_context(tc.tile_pool(name="gam", bufs=2))
        hbigp = ctx.enter_context(tc.tile_pool(name="hbig", bufs=2))
        hdp = ctx.enter_context(tc.tile_pool(name="hd", bufs=2))
        actp = ctx.enter_context(tc.tile_pool(name="act", bufs=2))
        osbp = ctx.enter_context(tc.tile_pool(name="osb", bufs=2))
        przp = ctx.enter_context(tc.tile_pool(name="prz", bufs=2, space="PSUM"))
        candp = ctx.enter_context(tc.tile_pool(name="cand", bufs=2, space="PSUM"))
        pjp = ctx.enter_context(tc.tile_pool(name="pj", bufs=2, space="PSUM"))

        wzr = constp.tile([128, KC * 2 * JT * 128], bf16)
        nc.sync.dma_start(wzr[:], wzr_d[:])
        wht = constp.tile([128, KC * JT * 128], bf16)
        nc.sync.dma_start(wht[:], wht_d[:])
        wlin = constp.tile([128, KC * JT * 128], bf16)
        nc.sync.dma_start(wlin[:], wlin_d[:])
        a0zr = constp.tile([128, 2 * FR], bf16)
        nc.sync.dma_start(a0zr[:], a0zr_d[:])
        a0h = constp.tile([128, FR], bf16)
        nc.sync.dma_start(a0h[:], a0h_d[:])
        ident = constp.tile([128, 128], bf16)
        nc.sync.dma_start(ident[:], ident_d[:])
        blinc = constp.tile([128, JT], f32)
        nc.sync.dma_start(blinc[:], blinc_d[:])

        def wzr_blk(g, jo, kc):
            i = ((kc * 2 + g) * JT + jo) * 128
            return wzr[:, i:i + 128]

        def wht_blk(jo, kc):
            i = (kc * JT + jo) * 128
            return wht[:, i:i + 128]

        def wlin_blk(jo, kc):
            i = (kc * JT + jo) * 128
            return wlin[:, i:i + 128]

        # gamma chunks, preloaded half a chunk ahead
        chunks = {}

        def ensure_chunk(c):
            if c in chunks or c * GCH >= T:
                return
            t0 = c * GCH
            t1 = min(t0 + GCH, T)
            gt = gpool.tile([128, GCH * FR], f32, tag="gchunk")
            nc.sync.dma_start(gt[:, 0:(t1 - t0) * FR], gam_d[:, t0:t1, :])
            chunks[c] = gt

        def gamma_sl(tt, c0, c1):
            c2, o2 = divmod(tt, GCH)
            return chunks[c2][:, o2 * FR + c0: o2 * FR + c1]

        ensure_chunk(0)

        # step-0 decayed state is zero
        hdf = hdp.tile([128, FR], f32, tag="hdf")
        nc.vector.memset(hdf[:], 0.0)
        hdb = hdp.tile([128, FR], bf16, tag="hdb")
        nc.vector.memset(hdb[:], 0.0)

        # ---- projection machinery ------------------------------------------
        hbig_tiles = {}        # chunk index -> archive tile [128, CH, FR]
        pj_tiles = {}          # (chunk, jo) -> psum tile
        pieces = []            # pending (chunk, jo, half) pieces

        def n_steps(c):
            return min(CH, T - c * CH)

        def emit_piece(c, jo, half):
            n = n_steps(c)
            if (c, jo) not in pj_tiles:
                pj_tiles[(c, jo)] = pjp.tile([128, CH, BL], f32, tag="pj",
                                             name=f"pj_{c}_{jo}")
            pj = pj_tiles[(c, jo)]
            hb = hbig_tiles[c]
            for kc in (2 * half, 2 * half + 1):
                nc.tensor.matmul(
                    pj[:, 0:n, :],
                    wlin_blk(jo, kc),
                    hb[:, 0:n, kc * BL:(kc + 1) * BL],
                    start=(kc == 0), stop=(kc == 3),
                )
            if half == 1:
                # bias add + PSUM drain on ACT (GpSimd cannot read PSUM)
                osb = osbp.tile([128, CH, BL], f32, tag="osb")
                nc.scalar.activation(
                    osb[:, 0:n, :], pj[:, 0:n, :],
                    AF.Identity, bias=blinc[:, jo:jo + 1])
                nc.sync.dma_start(
                    out_d[jo * 128:(jo + 1) * 128, c * CH:c * CH + n, :],
                    osb[:, 0:n, :])
                del pj_tiles[(c, jo)]

        for t in range(T):
            c, o = divmod(t, GCH)
            if o == GCH // 2:
                ensure_chunk(c + 1)

            pc, s = divmod(t, CH)
            if s == 0:
                hbig_tiles[pc] = hbigp.tile([128, CH, FR], bf16, tag="hbig",
                                            name=f"hbig_{pc}")

            # ---- dependency-free boundary filler: one projection piece
            if pieces:
                emit_piece(*pieces.pop(0))

            # ---- bias inits (depend only on constants + psum bank recycle)
            przt = przp.tile([128, 2 * FR], f32, tag="przt")
            nc.tensor.matmul(przt[:], ident[:], a0zr[:], start=True, stop=False)
            candt = candp.tile([128, FR], f32, tag="candt")
            nc.tensor.matmul(candt[:], ident[:], a0h[:], start=True, stop=False)

            # ---- r gate: (kc0,kc1) x all jo, then (kc2,kc3) by jo pairs so
            # sigmoid halves can start as soon as possible
            def gate_mm(g, jo, kc):
                nc.tensor.matmul(
                    przt[:, g * FR + jo * BL: g * FR + (jo + 1) * BL],
                    wzr_blk(g, jo, kc),
                    hdb[:, kc * BL:(kc + 1) * BL],
                    start=False, stop=(kc == KC - 1),
                )

            for kc in (0, 1):
                for jo in range(JT):
                    gate_mm(0, jo, kc)
            for jo in (0, 1):
                for kc in (2, 3):
                    gate_mm(0, jo, kc)
            rb = actp.tile([128, FR], bf16, tag="rb")
            nc.scalar.activation(rb[:, 0:HB], przt[:, 0:HB], AF.Sigmoid)
            for jo in (2, 3):
                for kc in (2, 3):
                    gate_mm(0, jo, kc)
            nc.scalar.activation(rb[:, HB:FR], przt[:, HB:FR], AF.Sigmoid)

            # ---- z gate (fills the PE while sigmoid(r)/rh run elsewhere)
            for kc in range(KC):
                for jo in range(JT):
                    gate_mm(1, jo, kc)
            zb = actp.tile([128, FR], bf16, tag="zb")

            rh = hdp.tile([128, FR], bf16, tag="rh")
            nc.vector.tensor_mul(rh[:, 0:HB], rb[:, 0:HB], hdb[:, 0:HB])
            nc.vector.tensor_mul(rh[:, HB:FR], rb[:, HB:FR], hdb[:, HB:FR])

            # ---- candidate: kc-major waves; half-0 (jo 0,1) stops first
            for kc in (0, 1):
                for jo in range(JT):
                    nc.tensor.matmul(
                        candt[:, jo * BL:(jo + 1) * BL],
                        wht_blk(jo, kc), rh[:, kc * BL:(kc + 1) * BL],
                        start=False, stop=False)
            for jo in (0, 1):
                for kc in (2, 3):
                    nc.tensor.matmul(
                        candt[:, jo * BL:(jo + 1) * BL],
                        wht_blk(jo, kc), rh[:, kc * BL:(kc + 1) * BL],
                        start=False, stop=(kc == 3))
            for jo in (2, 3):
                for kc in (2, 3):
                    nc.tensor.matmul(
                        candt[:, jo * BL:(jo + 1) * BL],
                        wht_blk(jo, kc), rh[:, kc * BL:(kc + 1) * BL],
                        start=False, stop=(kc == 3))

            nc.scalar.activation(zb[:], przt[:, FR:2 * FR], AF.Sigmoid)

            # ---- tail: htl = tanh(cand); h = hdec + z*(htl - hdec); decay
            htl = actp.tile([128, FR], f32, tag="htl")
            u = actp.tile([128, FR], f32, tag="u")
            v = actp.tile([128, FR], f32, tag="v")
            h = actp.tile([128, FR], f32, tag="h")
            hdf_n = hdb_n = None
            if t + 1 < T:
                hdf_n = hdp.tile([128, FR], f32, tag="hdf")
                hdb_n = hdp.tile([128, FR], bf16, tag="hdb")
            for hf in (0, 1):
                sl = slice(hf * HB, (hf + 1) * HB)
                nc.scalar.activation(htl[:, sl], candt[:, sl], AF.Tanh)
                nc.vector.tensor_sub(u[:, sl], htl[:, sl], hdf[:, sl])
                nc.vector.tensor_mul(v[:, sl], zb[:, sl], u[:, sl])
                nc.vector.tensor_add(h[:, sl], v[:, sl], hdf[:, sl])
                if t + 1 < T:
                    nc.vector.tensor_mul(
                        hdb_n[:, sl], gamma_sl(t + 1, hf * HB, (hf + 1) * HB), h[:, sl])
            if t + 1 < T:
                nc.vector.tensor_mul(hdf_n[:], gamma_sl(t + 1, 0, FR), h[:])
                hdf, hdb = hdf_n, hdb_n

            # archive h (bf16) for the chunked projection — on GpSimd
            nc.gpsimd.tensor_copy(hbig_tiles[pc][:, s, :], h[:])

            if s == CH - 1 or t == T - 1:
                for jo in range(JT):
                    pieces.append((pc, jo, 0))
                    pieces.append((pc, jo, 1))

        # drain remaining projection pieces
        while pieces:
            emit_piece(*pieces.pop(0))

    nc.compile()
    _BUILD_CACHE["nc"] = nc
    return nc


def _host_prep(C, t, Wz, bz, Wr, br, Wh, bh, Wgh, bgh, Wlin, blin):
    """Build per-core input maps (all the precomputed, packed device tensors)."""
    bf = ml_dtypes.bfloat16

    s = Wgh.sum(axis=0)  # (H,)
    t3 = t[:, :, 0]  # (T,B)
    dt = np.concatenate([np.zeros((1, B), np.float32), t3[1:] - t3[:-1]], axis=0)
    # gamma (T,B,H) fp32
    gam = np.exp(-np.maximum(dt[:, :, None] * s[None, None, :] + bgh[None, None, :], 0.0)).astype(np.float32)

    def gate_const(W, b):
        # C @ W_x + colsum(W_m) + b  -> (B,H)
        return C @ W[0:H] + (W[2 * H:3 * H].sum(axis=0) + b)[None, :]

    Az0 = gate_const(Wz, bz).astype(np.float32)
    Ar0 = gate_const(Wr, br).astype(np.float32)
    Ah0 = gate_const(Wh, bh).astype(np.float32)

    Wg = np.stack([Wr[H:2 * H], Wz[H:2 * H]])  # (2,H,H): g=0 -> r, g=1 -> z
    # wzr packed: [k, (kc,g,jo,m)]
    wzr = Wg.reshape(2, KC, 128, JT, 128).transpose(2, 1, 0, 3, 4).reshape(128, KC * 2 * JT * 128)
    wht = Wh[H:2 * H].reshape(KC, 128, JT, 128).transpose(1, 0, 2, 3).reshape(128, KC * JT * 128)
    # wlin packed: [k, (kc,jo,m)] with block (kc,jo) = Wlin[kc*128:(kc+1)*128, jo*128:(jo+1)*128]
    wlin = Wlin.reshape(KC, 128, JT, 128).transpose(1, 0, 2, 3).reshape(128, KC * JT * 128)
    wzr = np.ascontiguousarray(wzr, dtype=bf)
    wht = np.ascontiguousarray(wht, dtype=bf)
    wlin = np.ascontiguousarray(wlin, dtype=bf)
    ident = np.eye(128, dtype=bf)
    blinc = np.ascontiguousarray(blin.reshape(JT, 128).T, dtype=np.float32)  # [128, JT]

    in_maps = []
    for i in range(NCORES):
        sl = slice(i * BL, (i + 1) * BL)
        gf = gam[:, sl, :]  # (T,BL,H)
        # gam packed: [p, t, kt*BL+b]
        gp = np.ascontiguousarray(gf.reshape(T, BL, KC, 128).transpose(3, 0, 2, 1).reshape(128, T, KC * BL))

        def packA(A):
            return A[sl].reshape(BL, JT, 128).transpose(2, 1, 0).reshape(128, JT * BL)

        a0zr = np.concatenate([packA(Ar0), packA(Az0)], axis=1)

        in_maps.append({
            "gam": gp,
            "wzr": wzr,
            "wht": wht,
            "wlin": wlin,
            "a0zr": np.ascontiguousarray(a0zr, dtype=bf),
            "a0h": np.ascontiguousarray(packA(Ah0), dtype=bf),
            "ident": ident,
            "blinc": blinc,
        })
    return in_maps


def kernel(C, t, mask, Wz, bz, Wr, br, Wh, bh, Wgh, bgh, wgx, bgx, Wlin, blin,
           _trace=False, _trace_kwargs=None):
    C = np.asarray(C, np.float32)
    t = np.asarray(t, np.float32)
    nc = _build_program()
    in_maps = _host_prep(C, t,
                         np.asarray(Wz, np.float32), np.asarray(bz, np.float32),
                         np.asarray(Wr, np.float32), np.asarray(br, np.float32),
                         np.asarray(Wh, np.float32), np.asarray(bh, np.float32),
                         np.asarray(Wgh, np.float32), np.asarray(bgh, np.float32),
                         np.asarray(Wlin, np.float32), np.asarray(blin, np.float32))

    from concourse.bass_utils import run_bass_kernel_spmd
    res = run_bass_kernel_spmd(nc, in_maps, list(range(NCORES)),
                               trace=_trace, **(_trace_kwargs or {}))
    # per-core out is [O, T, BL] -> (T, BL, O)
    outs = [res.results[i]["out"].transpose(1, 2, 0) for i in range(NCORES)]
    full = np.concatenate(outs, axis=1).astype(np.float32)  # (T,B,O)
    kernel._last_results = res
    return full


# revision 7
# speedup vs baseline: 1.3080x; 1.1480x over previous
"""GRU-D decoder kernel for Trainium2 (8 NeuronCores, data-parallel over batch).

Math (mask == ones everywhere, which the reference hardcodes):
  x_hat = C (constant), d = dt broadcast, gamma_x unused.
  gamma[t,b,j] = exp(-relu(dt[t,b] * colsum(Wgh)[j] + bgh[j]))   (precomputed host-side)
  per step: hdec = gamma_t * h
            z = sigmoid(hdec @ Wz_h + Az0);  r = sigmoid(hdec @ Wr_h + Ar0)
            htl = tanh((r*hdec) @ Wh_h + Ah0)
            h = hdec + z*(htl - hdec)
  out[t] = h_t @ Wlin + blin
  where A?0 = C @ W?_x + colsum(W?_m) + b?  (time-constant, precomputed host-side).

Device layout: transposed (H on partitions as 4 tiles of 128, batch=64 on the
free dim), packed as SBUF tiles (128, 4*64) with column index = kt*64 + b.

v3 design:
  * The recurrent state is bf16 end-to-end (validated: global rel err 5e-3 vs
    the 2e-2 gate): gamma is shipped bf16, the decayed state hdb is the single
    state tensor, and the whole blend (u = htl - hdec, v = z*u, h = v + hdec)
    runs on DVE with all-bf16 operands for the 2x element rate.  h is written
    directly into the bf16 projection archive slot, so there is no separate
    fp32 h, no hdf, and no archive copy.
  * z and r share one PSUM bank [128,512] (r cols 0:256, z cols 256:512) with
    a single fused bias-init matmul (ident @ [a0r|a0z]).
  * The output projection is batched over 8-step chunks with wlin 128x128
    stationary blocks and 512-wide moving operands; pieces (2 matmuls) are
    drip-fed one per step boundary as dependency-free PE filler (after the
    bias matmuls so a stalled piece cannot block them).  The PSUM drain +
    blin add runs on DVE at the end of the step, off the critical path.
  * Gate matmul waves are ordered to minimize the recurrent chain: r runs
    (kc0,kc1) x all jo then (kc2,kc3) by jo pairs so sigmoid(r) halves start
    early; candidate runs kc-major with the half-0 stop first so tanh half 0
    starts before the last candidate matmuls finish.
  * Output is written [O, T, BL] per core (partition-major, 1KB DMA lines);
    the host transposes back.
"""

import numpy as np
import ml_dtypes

T, B, H, O = 100, 512, 512, 512
NCORES = 8
BL = B // NCORES  # 64
KC = 4   # contraction chunks of 128
JT = 4   # output j-tiles of 128
FR = JT * BL  # 256
HB = FR // 2  # 128
GCH = 20  # gamma chunk (steps per DMA)
CH = 8    # projection chunk (steps)

_BUILD_CACHE = {}


def _build_program():
    if "nc" in _BUILD_CACHE:
        return _BUILD_CACHE["nc"]

    import concourse.tile as tile
    import concourse.mybir as mybir
    from concourse import bacc
    from contextlib import ExitStack

    f32 = mybir.dt.float32
    bf16 = mybir.dt.bfloat16
    AF = mybir.ActivationFunctionType

    nc = bacc.Bacc("TRN2", target_bir_lowering=False, debug=False,
                   num_devices=NCORES)

    gam_d = nc.dram_tensor("gam", [128, T, FR], bf16, kind="ExternalInput")
    wzr_d = nc.dram_tensor("wzr", [128, KC * 2 * JT * 128], bf16, kind="ExternalInput")
    wht_d = nc.dram_tensor("wht", [128, KC * JT * 128], bf16, kind="ExternalInput")
    wlin_d = nc.dram_tensor("wlin", [128, KC * JT * 128], bf16, kind="ExternalInput")
    a0zr_d = nc.dram_tensor("a0zr", [128, 2 * FR], bf16, kind="ExternalInput")
    a0h_d = nc.dram_tensor("a0h", [128, FR], bf16, kind="ExternalInput")
    ident_d = nc.dram_tensor("ident", [128, 128], bf16, kind="ExternalInput")
    blinc_d = nc.dram_tensor("blinc", [128, JT], f32, kind="ExternalInput")
    out_d = nc.dram_tensor("out", [O, T, BL], f32, kind="ExternalOutput")

    with tile.TileContext(nc) as tc, ExitStack() as ctx:
        constp = ctx.enter_context(tc.tile_pool(name="const", bufs=1))
        gpool = ctx.enter_context(tc.tile_pool(name="gam", bufs=2))
        hbigp = ctx.enter_context(tc.tile_pool(name="hbig", bufs=2))
        hdp = ctx.enter_context(tc.tile_pool(name="hd", bufs=2))
        actp = ctx.enter_context(tc.tile_pool(name="act", bufs=2))
        osbp = ctx.enter_context(tc.tile_pool(name="osb", bufs=2))
        przp = ctx.enter_context(tc.tile_pool(name="prz", bufs=2, space="PSUM"))
        candp = ctx.enter_context(tc.tile_pool(name="cand", bufs=2, space="PSUM"))
        pjp = ctx.enter_context(tc.tile_pool(name="pj", bufs=3, space="PSUM"))

        wzr = constp.tile([128, KC * 2 * JT * 128], bf16)
        nc.sync.dma_start(wzr[:], wzr_d[:])
        wht = constp.tile([128, KC * JT * 128], bf16)
        nc.sync.dma_start(wht[:], wht_d[:])
        wlin = constp.tile([128, KC * JT * 128], bf16)
        nc.sync.dma_start(wlin[:], wlin_d[:])
        a0zr = constp.tile([128, 2 * FR], bf16)
        nc.sync.dma_start(a0zr[:], a0zr_d[:])
        a0h = constp.tile([128, FR], bf16)
        nc.sync.dma_start(a0h[:], a0h_d[:])
        ident = constp.tile([128, 128], bf16)
        nc.sync.dma_start(ident[:], ident_d[:])
        blinc = constp.tile([128, JT], f32)
        nc.sync.dma_start(blinc[:], blinc_d[:])

        def wzr_blk(g, jo, kc):
            i = ((kc * 2 + g) * JT + jo) * 128
            return wzr[:, i:i + 128]

        def wht_blk(jo, kc):
            i = (kc * JT + jo) * 128
            return wht[:, i:i + 128]

        def wlin_blk(jo, kc):
            i = (kc * JT + jo) * 128
            return wlin[:, i:i + 128]

        # gamma chunks, preloaded half a chunk ahead
        chunks = {}

        def ensure_chunk(c):
            if c in chunks or c * GCH >= T:
                return
            t0 = c * GCH
            t1 = min(t0 + GCH, T)
            gt = gpool.tile([128, GCH * FR], bf16, tag="gchunk")
            nc.sync.dma_start(gt[:, 0:(t1 - t0) * FR], gam_d[:, t0:t1, :])
            chunks[c] = gt

        def gamma_sl(tt, c0, c1):
            c2, o2 = divmod(tt, GCH)
            return chunks[c2][:, o2 * FR + c0: o2 * FR + c1]

        ensure_chunk(0)

        # step-0 decayed state is zero
        hdb = hdp.tile([128, FR], bf16, tag="hdb")
        nc.vector.memset(hdb[:], 0.0)

        # ---- projection machinery ------------------------------------------
        hbig_tiles = {}        # chunk index -> archive tile [128, CH, FR]
        pj_tiles = {}          # (chunk, jo) -> psum tile
        pieces = []            # pending (chunk, jo, half) matmul pieces
        drains = []            # pending (chunk, jo) PSUM drains

        def n_steps(c):
            return min(CH, T - c * CH)

        def emit_piece(c, jo, half):
            n = n_steps(c)
            if (c, jo) not in pj_tiles:
                pj_tiles[(c, jo)] = pjp.tile([128, CH, BL], f32, tag="pj",
                                             name=f"pj_{c}_{jo}")
            pj = pj_tiles[(c, jo)]
            hb = hbig_tiles[c]
            for kc in (2 * half, 2 * half + 1):
                nc.tensor.matmul(
                    pj[:, 0:n, :],
                    wlin_blk(jo, kc),
                    hb[:, 0:n, kc * BL:(kc + 1) * BL],
                    start=(kc == 0), stop=(kc == 3),
                )
            if half == 1:
                drains.append((c, jo))

        def emit_drain(c, jo):
            n = n_steps(c)
            pj = pj_tiles.pop((c, jo))
            osb = osbp.tile([128, CH, BL], f32, tag="osb")
            nc.vector.tensor_single_scalar(
                osb[:, 0:n, :], pj[:, 0:n, :],
                blinc[:, jo:jo + 1], mybir.AluOpType.add)
            nc.sync.dma_start(
                out_d[jo * 128:(jo + 1) * 128, c * CH:c * CH + n, :],
                osb[:, 0:n, :])

        for t in range(T):
            c, o = divmod(t, GCH)
            if o == GCH // 2:
                ensure_chunk(c + 1)

            pc, s = divmod(t, CH)
            if s == 0:
                hbig_tiles[pc] = hbigp.tile([128, CH, FR], bf16, tag="hbig",
                                            name=f"hbig_{pc}")
            hb = hbig_tiles[pc]

            # ---- bias inits first (depend only on constants + psum recycle)
            przt = przp.tile([128, 2 * FR], f32, tag="przt")
            nc.tensor.matmul(przt[:], ident[:], a0zr[:], start=True, stop=False)
            candt = candp.tile([128, FR], f32, tag="candt")
            nc.tensor.matmul(candt[:], ident[:], a0h[:], start=True, stop=False)

            # ---- dependency-free boundary filler: one projection piece
            if pieces:
                emit_piece(*pieces.pop(0))

            # ---- r gate: (kc0,kc1) x all jo, then (kc2,kc3) by jo pairs so
            # sigmoid halves can start as early as possible
            def gate_mm(g, jo, kc):
                nc.tensor.matmul(
                    przt[:, g * FR + jo * BL: g * FR + (jo + 1) * BL],
                    wzr_blk(g, jo, kc),
                    hdb[:, kc * BL:(kc + 1) * BL],
                    start=False, stop=(kc == KC - 1),
                )

            for kc in (0, 1):
                for jo in range(JT):
                    gate_mm(0, jo, kc)
            for jo in (0, 1):
                for kc in (2, 3):
                    gate_mm(0, jo, kc)
            rb = actp.tile([128, FR], bf16, tag="rb")
            nc.scalar.activation(rb[:, 0:HB], przt[:, 0:HB], AF.Sigmoid)
            for jo in (2, 3):
                for kc in (2, 3):
                    gate_mm(0, jo, kc)
            nc.scalar.activation(rb[:, HB:FR], przt[:, HB:FR], AF.Sigmoid)

            # ---- z gate (fills the PE while sigmoid(r)/rh run elsewhere)
            for kc in range(KC):
                for jo in range(JT):
                    gate_mm(1, jo, kc)
            zb = actp.tile([128, FR], bf16, tag="zb")

            rh = hdp.tile([128, FR], bf16, tag="rh")
            nc.vector.tensor_mul(rh[:, 0:HB], rb[:, 0:HB], hdb[:, 0:HB])
            nc.vector.tensor_mul(rh[:, HB:FR], rb[:, HB:FR], hdb[:, HB:FR])

            # ---- candidate: kc-major waves; half-0 (jo 0,1) stops first
            for kc in (0, 1):
                for jo in range(JT):
                    nc.tensor.matmul(
                        candt[:, jo * BL:(jo + 1) * BL],
                        wht_blk(jo, kc), rh[:, kc * BL:(kc + 1) * BL],
                        start=False, stop=False)
            for jo in (0, 1):
                for kc in (2, 3):
                    nc.tensor.matmul(
                        candt[:, jo * BL:(jo + 1) * BL],
                        wht_blk(jo, kc), rh[:, kc * BL:(kc + 1) * BL],
                        start=False, stop=(kc == 3))
            for jo in (2, 3):
                for kc in (2, 3):
                    nc.tensor.matmul(
                        candt[:, jo * BL:(jo + 1) * BL],
                        wht_blk(jo, kc), rh[:, kc * BL:(kc + 1) * BL],
                        start=False, stop=(kc == 3))

            nc.scalar.activation(zb[:], przt[:, FR:2 * FR], AF.Sigmoid)

            # ---- tail: htl = tanh(cand); h = hdec + z*(htl - hdec); decay.
            # All-bf16 on DVE (2x element rate); h lands in the archive slot.
            htl = actp.tile([128, FR], bf16, tag="htl")
            u = actp.tile([128, FR], bf16, tag="u")
            v = actp.tile([128, FR], bf16, tag="v")
            hdb_n = None
            if t + 1 < T:
                hdb_n = hdp.tile([128, FR], bf16, tag="hdb")
            for hf in (0, 1):
                sl = slice(hf * HB, (hf + 1) * HB)
                nc.scalar.activation(htl[:, sl], candt[:, sl], AF.Tanh)
                nc.vector.tensor_sub(u[:, sl], htl[:, sl], hdb[:, sl])
                nc.vector.tensor_mul(v[:, sl], zb[:, sl], u[:, sl])
                nc.vector.tensor_add(hb[:, s, sl], v[:, sl], hdb[:, sl])
                if t + 1 < T:
                    nc.vector.tensor_mul(
                        hdb_n[:, sl], gamma_sl(t + 1, hf * HB, (hf + 1) * HB),
                        hb[:, s, sl])
            if t + 1 < T:
                hdb = hdb_n

            if s == CH - 1 or t == T - 1:
                for jo in range(JT):
                    pieces.append((pc, jo, 0))
                    pieces.append((pc, jo, 1))

            # ---- PSUM drains for completed projection pieces (off-chain)
            while drains:
                emit_drain(*drains.pop(0))

        # drain remaining projection pieces
        while pieces:
            emit_piece(*pieces.pop(0))
        while drains:
            emit_drain(*drains.pop(0))

    nc.compile()
    _BUILD_CACHE["nc"] = nc
    return nc


def _host_prep(C, t, Wz, bz, Wr, br, Wh, bh, Wgh, bgh, Wlin, blin):
    """Build per-core input maps (all the precomputed, packed device tensors)."""
    bf = ml_dtypes.bfloat16

    s = Wgh.sum(axis=0)  # (H,)
    t3 = t[:, :, 0]  # (T,B)
    dt = np.concatenate([np.zeros((1, B), np.float32), t3[1:] - t3[:-1]], axis=0)
    # gamma (T,B,H)
    gam = np.exp(-np.maximum(dt[:, :, None] * s[None, None, :] + bgh[None, None, :], 0.0)).astype(np.float32)

    def gate_const(W, b):
        # C @ W_x + colsum(W_m) + b  -> (B,H)
        return C @ W[0:H] + (W[2 * H:3 * H].sum(axis=0) + b)[None, :]

    Az0 = gate_const(Wz, bz).astype(np.float32)
    Ar0 = gate_const(Wr, br).astype(np.float32)
    Ah0 = gate_const(Wh, bh).astype(np.float32)

    Wg = np.stack([Wr[H:2 * H], Wz[H:2 * H]])  # (2,H,H): g=0 -> r, g=1 -> z
    # wzr packed: [k, (kc,g,jo,m)]
    wzr = Wg.reshape(2, KC, 128, JT, 128).transpose(2, 1, 0, 3, 4).reshape(128, KC * 2 * JT * 128)
    wht = Wh[H:2 * H].reshape(KC, 128, JT, 128).transpose(1, 0, 2, 3).reshape(128, KC * JT * 128)
    # wlin packed: [k, (kc,jo,m)] with block (kc,jo) = Wlin[kc*128:(kc+1)*128, jo*128:(jo+1)*128]
    wlin = Wlin.reshape(KC, 128, JT, 128).transpose(1, 0, 2, 3).reshape(128, KC * JT * 128)
    wzr = np.ascontiguousarray(wzr, dtype=bf)
    wht = np.ascontiguousarray(wht, dtype=bf)
    wlin = np.ascontiguousarray(wlin, dtype=bf)
    ident = np.eye(128, dtype=bf)
    blinc = np.ascontiguousarray(blin.reshape(JT, 128).T, dtype=np.float32)  # [128, JT]

    in_maps = []
    for i in range(NCORES):
        sl = slice(i * BL, (i + 1) * BL)
        gf = gam[:, sl, :]  # (T,BL,H)
        # gam packed: [p, t, kt*BL+b]
        gp = np.ascontiguousarray(
            gf.reshape(T, BL, KC, 128).transpose(3, 0, 2, 1).reshape(128, T, KC * BL),
            dtype=bf)

        def packA(A):
            return A[sl].reshape(BL, JT, 128).transpose(2, 1, 0).reshape(128, JT * BL)

        a0zr = np.concatenate([packA(Ar0), packA(Az0)], axis=1)

        in_maps.append({
            "gam": gp,
            "wzr": wzr,
            "wht": wht,
            "wlin": wlin,
            "a0zr": np.ascontiguousarray(a0zr, dtype=bf),
            "a0h": np.ascontiguousarray(packA(Ah0), dtype=bf),
            "ident": ident,
            "blinc": blinc,
        })
    return in_maps


def kernel(C, t, mask, Wz, bz, Wr, br, Wh, bh, Wgh, bgh, wgx, bgx, Wlin, blin,
           _trace=False, _trace_kwargs=None):
    C = np.asarray(C, np.float32)
    t = np.asarray(t, np.float32)
    nc = _build_program()
    in_maps = _host_prep(C, t,
                         np.asarray(Wz, np.float32), np.asarray(bz, np.float32),
                         np.asarray(Wr, np.float32), np.asarray(br, np.float32),
                         np.asarray(Wh, np.float32), np.asarray(bh, np.float32),
                         np.asarray(Wgh, np.float32), np.asarray(bgh, np.float32),
                         np.asarray(Wlin, np.float32), np.asarray(blin, np.float32))

    from concourse.bass_utils import run_bass_kernel_spmd
    res = run_bass_kernel_spmd(nc, in_maps, list(range(NCORES)),
                               trace=_trace, **(_trace_kwargs or {}))
    # per-core out is [O, T, BL] -> (T, BL, O)
    outs = [res.results[i]["out"].transpose(1, 2, 0) for i in range(NCORES)]
    full = np.concatenate(outs, axis=1).astype(np.float32)  # (T,B,O)
    kernel._last_results = res
    return full


# revision 9
# speedup vs baseline: 1.5612x; 1.1935x over previous
"""GRU-D decoder kernel for Trainium2 (8 NeuronCores, data-parallel over batch).

Math (mask == ones everywhere, which the reference hardcodes):
  x_hat = C (constant), d = dt broadcast, gamma_x unused.
  gamma[t,b,j] = exp(-relu(dt[t,b] * colsum(Wgh)[j] + bgh[j]))   (precomputed host-side)
  per step: hdec = gamma_t * h
            z = sigmoid(hdec @ Wz_h + Az0);  r = sigmoid(hdec @ Wr_h + Ar0)
            htl = tanh((r*hdec) @ Wh_h + Ah0)
            h = hdec + z*(htl - hdec)
  out[t] = h_t @ Wlin + blin
  where A?0 = C @ W?_x + colsum(W?_m) + b?  (time-constant, precomputed host-side).

Device layout: transposed (H on partitions as 4 tiles of 128, batch=64 on the
free dim), packed as SBUF tiles (128, 4*64) with column index = kt*64 + b.

v3 design:
  * The recurrent state is bf16 end-to-end (validated: global rel err 5e-3 vs
    the 2e-2 gate): gamma is shipped bf16, the decayed state hdb is the single
    state tensor, and the whole blend (u = htl - hdec, v = z*u, h = v + hdec)
    runs on DVE with all-bf16 operands for the 2x element rate.  h is written
    directly into the bf16 projection archive slot, so there is no separate
    fp32 h, no hdf, and no archive copy.
  * z and r share one PSUM bank [128,512] (r cols 0:256, z cols 256:512) with
    a single fused bias-init matmul (ident @ [a0r|a0z]).
  * The output projection is batched over 8-step chunks with wlin 128x128
    stationary blocks and 512-wide moving operands; pieces (2 matmuls) are
    drip-fed one per step boundary as dependency-free PE filler (after the
    bias matmuls so a stalled piece cannot block them).  The PSUM drain +
    blin add runs on DVE at the end of the step, off the critical path.
  * Gate matmul waves are ordered to minimize the recurrent chain: r runs
    (kc0,kc1) x all jo then (kc2,kc3) by jo pairs so sigmoid(r) halves start
    early; candidate runs kc-major with the half-0 stop first so tanh half 0
    starts before the last candidate matmuls finish.
  * Output is written [O, T, BL] per core (partition-major, 1KB DMA lines);
    the host transposes back.
"""

import numpy as np
import ml_dtypes

T, B, H, O = 100, 512, 512, 512
NCORES = 8
BL = B // NCORES  # 64
KC = 4   # contraction chunks of 128
JT = 4   # output j-tiles of 128
FR = JT * BL  # 256
HB = FR // 2  # 128
GCH = 20  # gamma chunk (steps per DMA)
CH = 8    # projection chunk (steps)

_BUILD_CACHE = {}


def _build_program():
    if "nc" in _BUILD_CACHE:
        return _BUILD_CACHE["nc"]

    import concourse.tile as tile
    import concourse.mybir as mybir
    from concourse import bacc
    from contextlib import ExitStack

    f32 = mybir.dt.float32
    bf16 = mybir.dt.bfloat16
    AF = mybir.ActivationFunctionType

    nc = bacc.Bacc("TRN2", target_bir_lowering=False, debug=False,
                   num_devices=NCORES)

    gam_d = nc.dram_tensor("gam", [128, T, FR], bf16, kind="ExternalInput")
    wzr_d = nc.dram_tensor("wzr", [128, KC * 2 * JT * 128], bf16, kind="ExternalInput")
    wht_d = nc.dram_tensor("wht", [128, KC * JT * 128], bf16, kind="ExternalInput")
    wlin_d = nc.dram_tensor("wlin", [128, KC * JT * 128], bf16, kind="ExternalInput")
    a0zr_d = nc.dram_tensor("a0zr", [128, 2 * FR], bf16, kind="ExternalInput")
    a0h_d = nc.dram_tensor("a0h", [128, FR], bf16, kind="ExternalInput")
    ident_d = nc.dram_tensor("ident", [128, 128], bf16, kind="ExternalInput")
    blinc_d = nc.dram_tensor("blinc", [128, JT], f32, kind="ExternalInput")
    out_d = nc.dram_tensor("out", [O, T, BL], f32, kind="ExternalOutput")

    with tile.TileContext(nc) as tc, ExitStack() as ctx:
        constp = ctx.enter_context(tc.tile_pool(name="const", bufs=1))
        gpool = ctx.enter_context(tc.tile_pool(name="gam", bufs=2))
        hbigp = ctx.enter_context(tc.tile_pool(name="hbig", bufs=2))
        hdp = ctx.enter_context(tc.tile_pool(name="hd", bufs=2))
        actp = ctx.enter_context(tc.tile_pool(name="act", bufs=2))
        osbp = ctx.enter_context(tc.tile_pool(name="osb", bufs=2))
        prp = ctx.enter_context(tc.tile_pool(name="pr", bufs=2, space="PSUM"))
        pzp = ctx.enter_context(tc.tile_pool(name="pz", bufs=2, space="PSUM"))
        candp = ctx.enter_context(tc.tile_pool(name="cand", bufs=2, space="PSUM"))
        pjp = ctx.enter_context(tc.tile_pool(name="pj", bufs=2, space="PSUM"))

        wzr = constp.tile([128, KC * 2 * JT * 128], bf16)
        nc.sync.dma_start(wzr[:], wzr_d[:])
        wht = constp.tile([128, KC * JT * 128], bf16)
        nc.sync.dma_start(wht[:], wht_d[:])
        wlin = constp.tile([128, KC * JT * 128], bf16)
        nc.sync.dma_start(wlin[:], wlin_d[:])
        a0zr = constp.tile([128, 2 * FR], bf16)
        nc.sync.dma_start(a0zr[:], a0zr_d[:])
        a0h = constp.tile([128, FR], bf16)
        nc.sync.dma_start(a0h[:], a0h_d[:])
        ident = constp.tile([128, 128], bf16)
        nc.sync.dma_start(ident[:], ident_d[:])
        blinc = constp.tile([128, JT], f32)
        nc.sync.dma_start(blinc[:], blinc_d[:])

        def wzr_blk(g, jo, kc):
            i = ((kc * 2 + g) * JT + jo) * 128
            return wzr[:, i:i + 128]

        def wht_blk(jo, kc):
            i = (kc * JT + jo) * 128
            return wht[:, i:i + 128]

        def wlin_blk(jo, kc):
            i = (kc * JT + jo) * 128
            return wlin[:, i:i + 128]

        # gamma chunks, preloaded half a chunk ahead
        chunks = {}

        def ensure_chunk(c):
            if c in chunks or c * GCH >= T:
                return
            t0 = c * GCH
            t1 = min(t0 + GCH, T)
            gt = gpool.tile([128, GCH * FR], bf16, tag="gchunk")
            nc.sync.dma_start(gt[:, 0:(t1 - t0) * FR], gam_d[:, t0:t1, :])
            chunks[c] = gt

        def gamma_sl(tt, c0, c1):
            c2, o2 = divmod(tt, GCH)
            return chunks[c2][:, o2 * FR + c0: o2 * FR + c1]

        ensure_chunk(0)

        # step-0 decayed state is zero
        hdb = hdp.tile([128, FR], bf16, tag="hdb")
        nc.vector.memset(hdb[:], 0.0)

        # ---- projection machinery ------------------------------------------
        hbig_tiles = {}        # chunk index -> archive tile [128, CH, FR]
        pj_tiles = {}          # (chunk, jo) -> psum tile
        pieces = []            # pending (chunk, jo, half) matmul pieces
        drains = []            # pending (chunk, jo) PSUM drains

        def n_steps(c):
            return min(CH, T - c * CH)

        def emit_piece(c, jo, half):
            n = n_steps(c)
            if (c, jo) not in pj_tiles:
                pj_tiles[(c, jo)] = pjp.tile([128, CH, BL], f32, tag="pj",
                                             name=f"pj_{c}_{jo}")
            pj = pj_tiles[(c, jo)]
            hb = hbig_tiles[c]
            for kc in (2 * half, 2 * half + 1):
                nc.tensor.matmul(
                    pj[:, 0:n, :],
                    wlin_blk(jo, kc),
                    hb[:, 0:n, kc * BL:(kc + 1) * BL],
                    start=(kc == 0), stop=(kc == 3),
                )
            if half == 1:
                drains.append((c, jo))

        def emit_drain(c, jo):
            n = n_steps(c)
            pj = pj_tiles.pop((c, jo))
            osb = osbp.tile([128, CH, BL], f32, tag="osb")
            nc.vector.tensor_single_scalar(
                osb[:, 0:n, :], pj[:, 0:n, :],
                blinc[:, jo:jo + 1], mybir.AluOpType.add)
            nc.sync.dma_start(
                out_d[jo * 128:(jo + 1) * 128, c * CH:c * CH + n, :],
                osb[:, 0:n, :])

        for t in range(T):
            c, o = divmod(t, GCH)
            if o == GCH // 2:
                ensure_chunk(c + 1)

            pc, s = divmod(t, CH)
            if s == 0:
                hbig_tiles[pc] = hbigp.tile([128, CH, FR], bf16, tag="hbig",
                                            name=f"hbig_{pc}")
            hb = hbig_tiles[pc]

            # ---- bias inits first (depend only on constants + psum recycle)
            pr = prp.tile([128, FR], f32, tag="pr")
            nc.tensor.matmul(pr[:], ident[:], a0zr[:, 0:FR], start=True, stop=False)
            pz = pzp.tile([128, FR], f32, tag="pz")
            nc.tensor.matmul(pz[:], ident[:], a0zr[:, FR:2 * FR], start=True, stop=False)
            candt = candp.tile([128, FR], f32, tag="candt")
            nc.tensor.matmul(candt[:], ident[:], a0h[:], start=True, stop=False)

            # ---- dependency-free boundary filler: one projection piece.
            # A chunk's pieces start one step after the chunk completes so the
            # first piece never waits on the last archive-slot write.
            if pieces and t >= (pieces[0][0] + 1) * CH + 1:
                emit_piece(*pieces.pop(0))

            # ---- r gate: (kc0,kc1) x all jo, then (kc2,kc3) by jo pairs so
            # sigmoid halves can start as early as possible
            def gate_mm(g, jo, kc):
                tgt = pr if g == 0 else pz
                nc.tensor.matmul(
                    tgt[:, jo * BL:(jo + 1) * BL],
                    wzr_blk(g, jo, kc),
                    hdb[:, kc * BL:(kc + 1) * BL],
                    start=False, stop=(kc == KC - 1),
                )

            for kc in (0, 1):
                for jo in range(JT):
                    gate_mm(0, jo, kc)
            for jo in (0, 1):
                for kc in (2, 3):
                    gate_mm(0, jo, kc)
            rb = actp.tile([128, FR], bf16, tag="rb")
            nc.scalar.activation(rb[:, 0:HB], pr[:, 0:HB], AF.Sigmoid)
            for jo in (2, 3):
                for kc in (2, 3):
                    gate_mm(0, jo, kc)
            nc.scalar.activation(rb[:, HB:FR], pr[:, HB:FR], AF.Sigmoid)

            # ---- z gate (fills the PE while sigmoid(r)/rh run elsewhere)
            for kc in range(KC):
                for jo in range(JT):
                    gate_mm(1, jo, kc)
            zb = actp.tile([128, FR], bf16, tag="zb")

            rh = hdp.tile([128, FR], bf16, tag="rh")
            nc.vector.tensor_mul(rh[:, 0:HB], rb[:, 0:HB], hdb[:, 0:HB])
            nc.vector.tensor_mul(rh[:, HB:FR], rb[:, HB:FR], hdb[:, HB:FR])

            # ---- candidate: kc-major waves; half-0 (jo 0,1) stops first
            for kc in (0, 1):
                for jo in range(JT):
                    nc.tensor.matmul(
                        candt[:, jo * BL:(jo + 1) * BL],
                        wht_blk(jo, kc), rh[:, kc * BL:(kc + 1) * BL],
                        start=False, stop=False)
            for jo in (0, 1):
                for kc in (2, 3):
                    nc.tensor.matmul(
                        candt[:, jo * BL:(jo + 1) * BL],
                        wht_blk(jo, kc), rh[:, kc * BL:(kc + 1) * BL],
                        start=False, stop=(kc == 3))
            for jo in (2, 3):
                for kc in (2, 3):
                    nc.tensor.matmul(
                        candt[:, jo * BL:(jo + 1) * BL],
                        wht_blk(jo, kc), rh[:, kc * BL:(kc + 1) * BL],
                        start=False, stop=(kc == 3))

            nc.scalar.activation(zb[:], pz[:], AF.Sigmoid)

            # ---- tail: h = z*htl + (hdec - z*hdec).  The second term is
            # precomputed off the tanh critical path; all-bf16 DVE ops.
            pq = actp.tile([128, FR], bf16, tag="pq")
            nc.vector.tensor_mul(pq[:], zb[:], hdb[:])
            gg = actp.tile([128, FR], bf16, tag="gg")
            nc.vector.tensor_sub(gg[:], hdb[:], pq[:])
            htl = actp.tile([128, FR], bf16, tag="htl")
            qq = actp.tile([128, FR], bf16, tag="qq")
            hdb_n = None
            if t + 1 < T:
                hdb_n = hdp.tile([128, FR], bf16, tag="hdb")
            for hf in (0, 1):
                sl = slice(hf * HB, (hf + 1) * HB)
                nc.scalar.activation(htl[:, sl], candt[:, sl], AF.Tanh)
                nc.vector.tensor_mul(qq[:, sl], zb[:, sl], htl[:, sl])
                nc.vector.tensor_add(hb[:, s, sl], qq[:, sl], gg[:, sl])
                if t + 1 < T:
                    nc.vector.tensor_mul(
                        hdb_n[:, sl], gamma_sl(t + 1, hf * HB, (hf + 1) * HB),
                        hb[:, s, sl])
            if t + 1 < T:
                hdb = hdb_n

            if s == CH - 1 or t == T - 1:
                for jo in range(JT):
                    pieces.append((pc, jo, 0))
                    pieces.append((pc, jo, 1))

            # ---- PSUM drains for completed projection pieces (off-chain)
            while drains:
                emit_drain(*drains.pop(0))

        # drain remaining projection pieces
        while pieces:
            emit_piece(*pieces.pop(0))
        while drains:
            emit_drain(*drains.pop(0))

    nc.compile()
    _BUILD_CACHE["nc"] = nc
    return nc


def _host_prep(C, t, Wz, bz, Wr, br, Wh, bh, Wgh, bgh, Wlin, blin):
    """Build per-core input maps (all the precomputed, packed device tensors)."""
    bf = ml_dtypes.bfloat16

    s = Wgh.sum(axis=0)  # (H,)
    t3 = t[:, :, 0]  # (T,B)
    dt = np.concatenate([np.zeros((1, B), np.float32), t3[1:] - t3[:-1]], axis=0)
    # gamma (T,B,H)
    gam = np.exp(-np.maximum(dt[:, :, None] * s[None, None, :] + bgh[None, None, :], 0.0)).astype(np.float32)

    def gate_const(W, b):
        # C @ W_x + colsum(W_m) + b  -> (B,H)
        return C @ W[0:H] + (W[2 * H:3 * H].sum(axis=0) + b)[None, :]

    Az0 = gate_const(Wz, bz).astype(np.float32)
    Ar0 = gate_const(Wr, br).astype(np.float32)
    Ah0 = gate_const(Wh, bh).astype(np.float32)

    Wg = np.stack([Wr[H:2 * H], Wz[H:2 * H]])  # (2,H,H): g=0 -> r, g=1 -> z
    # wzr packed: [k, (kc,g,jo,m)]
    wzr = Wg.reshape(2, KC, 128, JT, 128).transpose(2, 1, 0, 3, 4).reshape(128, KC * 2 * JT * 128)
    wht = Wh[H:2 * H].reshape(KC, 128, JT, 128).transpose(1, 0, 2, 3).reshape(128, KC * JT * 128)
    # wlin packed: [k, (kc,jo,m)] with block (kc,jo) = Wlin[kc*128:(kc+1)*128, jo*128:(jo+1)*128]
    wlin = Wlin.reshape(KC, 128, JT, 128).transpose(1, 0, 2, 3).reshape(128, KC * JT * 128)
    wzr = np.ascontiguousarray(wzr, dtype=bf)
    wht = np.ascontiguousarray(wht, dtype=bf)
    wlin = np.ascontiguousarray(wlin, dtype=bf)
    ident = np.eye(128, dtype=bf)
    blinc = np.ascontiguousarray(blin.reshape(JT, 128).T, dtype=np.float32)  # [128, JT]

    in_maps = []
    for i in range(NCORES):
        sl = slice(i * BL, (i + 1) * BL)
        gf = gam[:, sl, :]  # (T,BL,H)
        # gam packed: [p, t, kt*BL+b]
        gp = np.ascontiguousarray(
            gf.reshape(T, BL, KC, 128).transpose(3, 0, 2, 1).reshape(128, T, KC * BL),
            dtype=bf)

        def packA(A):
            return A[sl].reshape(BL, JT, 128).transpose(2, 1, 0).reshape(128, JT * BL)

        a0zr = np.concatenate([packA(Ar0), packA(Az0)], axis=1)

        in_maps.append({
            "gam": gp,
            "wzr": wzr,
            "wht": wht,
            "wlin": wlin,
            "a0zr": np.ascontiguousarray(a0zr, dtype=bf),
            "a0h": np.ascontiguousarray(packA(Ah0), dtype=bf),
            "ident": ident,
            "blinc": blinc,
        })
    return in_maps


def kernel(C, t, mask, Wz, bz, Wr, br, Wh, bh, Wgh, bgh, wgx, bgx, Wlin, blin,
           _trace=False, _trace_kwargs=None):
    C = np.asarray(C, np.float32)
    t = np.asarray(t, np.float32)
    nc = _build_program()
    in_maps = _host_prep(C, t,
                         np.asarray(Wz, np.float32), np.asarray(bz, np.float32),
                         np.asarray(Wr, np.float32), np.asarray(br, np.float32),
                         np.asarray(Wh, np.float32), np.asarray(bh, np.float32),
                         np.asarray(Wgh, np.float32), np.asarray(bgh, np.float32),
                         np.asarray(Wlin, np.float32), np.asarray(blin, np.float32))

    from concourse.bass_utils import run_bass_kernel_spmd
    res = run_bass_kernel_spmd(nc, in_maps, list(range(NCORES)),
                               trace=_trace, **(_trace_kwargs or {}))
    # per-core out is [O, T, BL] -> (T, BL, O)
    outs = [res.results[i]["out"].transpose(1, 2, 0) for i in range(NCORES)]
    full = np.concatenate(outs, axis=1).astype(np.float32)  # (T,B,O)
    kernel._last_results = res
    return full
